# revision 8
# baseline (speedup 1.0000x reference)
"""Fused DeepFeatureLoss kernel for 8 Trainium2 NeuronCores (v2).

Reference computation (per batch b, N=4096 points, D=32 features):
    pd[i,j] = -||p_i - p_j||^2 / sigma^2          (points, sigma=0.005)
    fd[i,j] = -||f1_i - f2_j||^2
    ce[i]   = -sum_j softmax(pd)[i,j] * log_softmax(fd)[i,j]
    ce_loss[b]  = sum_i ce[i] * w[i]
    reg_loss[b] = mean_{i, c>=3} (f1[i,c]^2 + f2[i,c]^2)

Identity: ce[i] = ln(Zf_i) - S_i/Zp_i with
    Zf_i = sum_j exp(fd[i,j]);  Zp_i = sum_j exp(pd[i,j]);  S_i = sum_j exp(pd)*fd.

Device work per core (batch k//4, rows [1024*(k%4), +1024), 8 blocks of 128):
  - fd is produced directly in PSUM pre-scaled: the augmented K=35 matmul
    computes  v = a*fd + bconst  with a = 184 (bf16-exact; the features are
    pre-scaled by sqrt((128/ln2)/184) on the host so a acts as the exact
    Schraudolph constant 128/ln2) and bconst = 127*128 = 16256.
  - ACT chunks (c in ACT_CHUNKS): exp via the activation's free affine
    (scale=1/a, bias=-bconst/a restores fd exactly), written back in place,
    row sum via the ACT accumulator -> zfa.
  - DVE chunks: Schraudolph exp = bitcast_bf16(int16(max(v, 0))).  One
    tensor_scalar converts PSUM fp32 -> int16 SBUF with clamp; Pool (or DVE)
    row-sums the bf16-bitcast view -> zfs.  The host divides by the
    calibrated staircase mean KAPPA.
  - Gaussian band (W=384 cols around the diagonal in Morton order, exact
    permutation as in the baseline): pd band matmul fp32, fd band matmul
    fp32r; ep = exp(pd) on ACT (bf16); Zp via Pool row-sum; S via DVE
    scalar_tensor_tensor accum.
  - Output: 48 partial columns [zfa(16) zfs(16) zp(8) s(8)] per core.
Host: Zf = zfa + zfs/KAPPA, ce = w*(ln Zf - S/Zp) summed; reg_loss directly
from f1/f2 (it only needs the per-row squares already formed during operand
prep).  ln/reg on the host are O(N) postprocessing of device reductions.
"""

import math

import ml_dtypes
import numpy as np
from contextlib import ExitStack

import concourse.bacc as bacc
import concourse.bass as bass
import concourse.tile as tile
from concourse import mybir
from concourse.bass_utils import run_bass_kernel_spmd

SIGMA = 0.005
B, N, D = 2, 4096, 32
NCORES = 8
CPB = NCORES // B            # cores per batch = 4
ROWS = N // CPB              # rows per core = 1024
RB = ROWS // 128             # 128-row blocks per core = 8
NFC = 4                      # fd chunks per row block
FCH = N // NFC               # fd chunk width = 1024 (2 PSUM banks)
W = 384                      # point-band width
PAD = (W - 128) // 2         # 128
KP = 5                       # augmented K for points
KB = D + 2                   # augmented K for band features = 34
KS = D + 3                   # augmented K for scaled fd = 35
F32 = mybir.dt.float32
F32R = mybir.dt.float32r
BF16 = mybir.dt.bfloat16
I16 = mybir.dt.int16

A_TRUE = 128.0 / math.log(2.0)        # 184.6649652337873
A_USED = 184.0                        # bf16-exact
CSCALE = math.sqrt(A_TRUE / A_USED)   # host feature pre-scale
BCONST = 16256.0                      # 127 * 128, bf16-exact
# mean of the Schraudolph staircase ratio approx/exp on [-18, 0] with
# round-to-nearest; recalibrated against HW after the first run.
KAPPA = 1.039720

ACT_CHUNKS = (0, 1)                   # chunks per block on ACT; rest on DVE
DVE_CHUNKS = tuple(c for c in range(NFC) if c not in ACT_CHUNKS)
POOL_REDUCE = False                   # Pool rejects TensorScalarPtrReduce; DVE 4x it is

_CACHE = {}


def _build():
    nc = bacc.Bacc(trn_type="TRN2")
    afs = nc.declare_dram_parameter("afs", [KS, ROWS], BF16, isOutput=False)
    bfs = nc.declare_dram_parameter("bfs", [KS, N], BF16, isOutput=False)
    apt = nc.declare_dram_parameter("apt", [KP, ROWS], F32, isOutput=False)
    bpt = nc.declare_dram_parameter("bpt", [KP, RB * W], F32, isOutput=False)
    afr = nc.declare_dram_parameter("afr", [KB, ROWS], F32R, isOutput=False)
    bfb = nc.declare_dram_parameter("bfb", [KB, RB * W], F32R, isOutput=False)
    outp = nc.declare_dram_parameter("partials", [128, 48], F32, isOutput=True)

    AF = mybir.ActivationFunctionType
    OP = mybir.AluOpType

    with ExitStack() as ctx:
        tc = ctx.enter_context(tile.TileContext(nc))
        singles = ctx.enter_context(tc.tile_pool(name="singles", bufs=1))
        fd_pool = ctx.enter_context(tc.tile_pool(name="fdp", bufs=3, space="PSUM"))
        pdb_pool = ctx.enter_context(tc.tile_pool(name="pdbp", bufs=1, space="PSUM"))
        fdb_pool = ctx.enter_context(tc.tile_pool(name="fdbp", bufs=1, space="PSUM"))
        e16_pool = ctx.enter_context(tc.tile_pool(name="e16p", bufs=2))
        scr_pool = ctx.enter_context(tc.tile_pool(name="scrp", bufs=2))
        ep_pool = ctx.enter_context(tc.tile_pool(name="epp", bufs=2))
        es_pool = ctx.enter_context(tc.tile_pool(name="esp", bufs=2))
        ss_pool = ctx.enter_context(tc.tile_pool(name="ssp", bufs=2))

        # ---- input loads, spread across engine queues so issue overlaps ----
        afs_sb = singles.tile([128, ROWS], BF16)
        bfs_sb = singles.tile([128, N], BF16)
        # SP queue: group-0 copies (first compute operands)
        nc.sync.dma_start(out=afs_sb[0:KS, :], in_=afs[:, :])
        nc.sync.dma_start(out=bfs_sb[0:KS, 0:2048], in_=bfs[:, 0:2048])
        nc.sync.dma_start(out=bfs_sb[0:KS, 2048:4096], in_=bfs[:, 2048:4096])
        # ACT queue: group-64 copies
        nc.scalar.dma_start(out=afs_sb[64 : 64 + KS, :], in_=afs[:, :])
        nc.scalar.dma_start(out=bfs_sb[64 : 64 + KS, 0:2048], in_=bfs[:, 0:2048])
        nc.scalar.dma_start(out=bfs_sb[64 : 64 + KS, 2048:4096], in_=bfs[:, 2048:4096])
        # Pool SWDGE: point band operands (both pdb row groups); cheap issue
        apt_sb = singles.tile([128, ROWS], F32)
        bpt_sb = singles.tile([128, RB * W], F32)
        nc.gpsimd.dma_start(out=apt_sb[96 : 96 + KP, :], in_=apt[:, :])
        nc.gpsimd.dma_start(out=bpt_sb[96 : 96 + KP, :], in_=bpt[:, :])
        nc.gpsimd.dma_start(out=apt_sb[64 : 64 + KP, :], in_=apt[:, :])
        nc.gpsimd.dma_start(out=bpt_sb[64 : 64 + KP, :], in_=bpt[:, :])
        # Pool SWDGE: band feature operands (cheap queue issue)
        afr_sb = singles.tile([KB, ROWS], F32R)
        bfb_sb = singles.tile([KB, RB * W], F32R)
        nc.gpsimd.dma_start(out=afr_sb[:, :], in_=afr[:, :])
        nc.gpsimd.dma_start(out=bfb_sb[:, :], in_=bfb[:, :])

        bias_sb = singles.tile([128, 1], F32)
        nc.vector.memset(bias_sb, -BCONST / A_TRUE)

        out_sb = singles.tile([128, 48], F32)
        zfa = out_sb[:, 0:16]
        zfs = out_sb[:, 16:32]
        zp = out_sb[:, 32:40]
        s_ = out_sb[:, 40:48]

        for rb in range(RB):
            r0 = rb * 128
            fdts = []
            for c in range(NFC):
                fdt = fd_pool.tile([128, FCH], F32, tag="fdt", name=f"fd_{rb}_{c}")
                j0 = c * FCH
                nc.tensor.matmul(
                    fdt[:, 0:512],
                    lhsT=afs_sb[0:KS, r0 : r0 + 128],
                    rhs=bfs_sb[0:KS, j0 : j0 + 512],
                    start=True,
                    stop=True,
                )
                nc.tensor.matmul(
                    fdt[:, 512:1024],
                    lhsT=afs_sb[64 : 64 + KS, r0 : r0 + 128],
                    rhs=bfs_sb[64 : 64 + KS, j0 + 512 : j0 + 1024],
                    start=True,
                    stop=True,
                    tile_position=(64, 0),
                )
                fdts.append(fdt)
                if c in ACT_CHUNKS:
                    # exact exp: undo the a*fd+b affine via ACT's free affine
                    nc.scalar.activation(
                        out=fdt[:, :],
                        in_=fdt[:, :],
                        func=AF.Exp,
                        scale=1.0 / A_TRUE,
                        bias=bias_sb[:, 0:1],
                        accum_out=zfa[:, rb * 2 + ACT_CHUNKS.index(c) : rb * 2 + ACT_CHUNKS.index(c) + 1],
                    )
                else:
                    di = DVE_CHUNKS.index(c)
                    e16 = e16_pool.tile([128, FCH], I16, tag="e16")
                    nc.vector.tensor_scalar(
                        out=e16[:, :],
                        in0=fdt[:, :],
                        scalar1=0.0,
                        scalar2=None,
                        op0=OP.max,
                    )
                    scr = scr_pool.tile([128, FCH], BF16, tag="scr")
                    eng = nc.gpsimd if POOL_REDUCE else nc.vector
                    eng.tensor_scalar(
                        out=scr[:, :],
                        in0=e16[:, :].bitcast(BF16),
                        scalar1=1.0,
                        scalar2=0.0,
                        op0=OP.mult,
                        op1=OP.add,
                        accum_out=zfs[:, rb * 2 + di : rb * 2 + di + 1],
                    )
            # ---- band ----
            fdbt = fdb_pool.tile([128, W], F32, tag="fdbt", name=f"fdb_{rb}")
            nc.tensor.matmul(
                fdbt[:, :],
                lhsT=afr_sb[0:KB, r0 : r0 + 128],
                rhs=bfb_sb[0:KB, rb * W : (rb + 1) * W],
                start=True,
                stop=True,
            )
            pb = 96 if rb % 2 == 0 else 64
            pdbt = pdb_pool.tile([128, W], F32, tag="pdbt", name=f"pdb_{rb}")
            nc.tensor.matmul(
                pdbt[:, :],
                lhsT=apt_sb[pb : pb + KP, r0 : r0 + 128],
                rhs=bpt_sb[pb : pb + KP, rb * W : (rb + 1) * W],
                start=True,
                stop=True,
                tile_position=(pb, 0),
            )
            ep = ep_pool.tile([128, W], BF16, tag="ep")
            nc.scalar.activation(out=ep, in_=pdbt[:, :], func=AF.Exp)
            es = es_pool.tile([128, W], BF16, tag="es")
            eng = nc.gpsimd if POOL_REDUCE else nc.vector
            eng.tensor_scalar(
                out=es[:, :],
                in0=ep[:, :],
                scalar1=1.0,
                scalar2=0.0,
                op0=OP.mult,
                op1=OP.add,
                accum_out=zp[:, rb : rb + 1],
            )
            sscr = ss_pool.tile([128, W], BF16, tag="sscr")
            nc.vector.scalar_tensor_tensor(
                out=sscr,
                in0=fdbt[:, :],
                scalar=1.0,
                in1=ep[:, :],
                op0=OP.mult,
                op1=OP.mult,
                accum_out=s_[:, rb : rb + 1],
            )

        nc.sync.dma_start(out=outp[:, :], in_=out_sb[:, :])
    return nc


def _morton(p, bits=10):
    q = np.minimum((p * (1 << bits)).astype(np.uint64), (1 << bits) - 1)
    code = np.zeros(len(p), np.uint64)
    for b in range(bits):
        for dim in range(3):
            code |= ((q[:, dim] >> np.uint64(b)) & np.uint64(1)) << np.uint64(3 * b + dim)
    return code


def _fp22(x):
    return (x.view(np.uint32) & np.uint32(0xFFFFFC00)).view(np.float32)


def _prep_batch(b, points, pointfea1, pointfea2, weights):
    perm = np.argsort(_morton(points[b]))
    inv = np.float32(1.0 / (SIGMA * SIGMA))
    p = points[b][perm]
    f1 = pointfea1[b][perm]
    f2 = pointfea2[b][perm]
    w = weights[b, :, 0][perm]

    p2 = (p * p).sum(1)
    onesN = np.ones((N, 1), np.float32)

    a_pts = np.concatenate([2.0 * p * inv, onesN, (p2 * inv)[:, None]], 1).astype(np.float32)
    b_pts = np.concatenate([p, -(p2 * inv)[:, None], -onesN], 1).astype(np.float32)

    # band (unscaled) feature operands, fp22 for fp32r matmuls
    f1sq = (f1 * f1).sum(1)
    f2sq = (f2 * f2).sum(1)
    a_fea = _fp22(np.concatenate([2.0 * f1, onesN, f1sq[:, None]], 1).astype(np.float32))
    b_fea = _fp22(np.concatenate([f2, -f2sq[:, None], -onesN], 1).astype(np.float32))

    # scaled fd operands (Schraudolph units), bf16
    c = np.float32(CSCALE)
    f1c = c * f1
    f2c = c * f2
    f1csq = (f1c * f1c).sum(1)
    f2csq = (f2c * f2c).sum(1)
    au = np.float32(A_USED)
    a_s = np.concatenate(
        [2.0 * au * f1c, au * onesN, (au * f1csq)[:, None], onesN], 1
    ).astype(ml_dtypes.bfloat16)
    b_s = np.concatenate(
        [f2c, -f2csq[:, None], -onesN, np.float32(BCONST) * onesN], 1
    ).astype(ml_dtypes.bfloat16)
    return p, f1, f2, w, a_pts, b_pts, a_fea, b_fea, a_s, b_s


def make_in_maps(points, pointfea1, pointfea2, weights):
    points = np.asarray(points, np.float32)
    pointfea1 = np.asarray(pointfea1, np.float32)
    pointfea2 = np.asarray(pointfea2, np.float32)
    weights = np.asarray(weights, np.float32)

    batch_data = [
        _prep_batch(b, points, pointfea1, pointfea2, weights) for b in range(B)
    ]
    in_maps = []
    for k in range(NCORES):
        b = k // CPB
        r0 = (k % CPB) * ROWS
        p, f1, f2, w, a_pts, b_pts, a_fea, b_fea, a_s, b_s = batch_data[b]
        bpt_band = np.empty((KP, RB * W), np.float32)
        bfb_band = np.empty((KB, RB * W), np.float32)
        for rb in range(RB):
            g0 = r0 + rb * 128
            s = min(max(g0 - PAD, 0), N - W)
            bpt_band[:, rb * W : (rb + 1) * W] = b_pts[s : s + W].T
            bfb_band[:, rb * W : (rb + 1) * W] = b_fea[s : s + W].T
        in_maps.append(
            {
                "afs": np.ascontiguousarray(a_s[r0 : r0 + ROWS].T),
                "bfs": np.ascontiguousarray(b_s.T),
                "apt": np.ascontiguousarray(a_pts[r0 : r0 + ROWS].T),
                "bpt": bpt_band,
                "afr": np.ascontiguousarray(a_fea[r0 : r0 + ROWS].T),
                "bfb": bfb_band,
            }
        )
    return in_maps


def get_nc():
    if "nc" not in _CACHE:
        nc = _build()
        nc.finalize()
        _CACHE["nc"] = nc
    return _CACHE["nc"]


def combine_partials(parts, points, pointfea1, pointfea2, weights):
    """parts: [NCORES, 128, 48]. Host: Zf assembly, ln, ce sum, reg."""
    parts = np.asarray(parts, np.float64)
    points = np.asarray(points, np.float32)
    weights = np.asarray(weights, np.float32)
    ce = np.zeros(B, np.float64)
    for k in range(NCORES):
        b = k // CPB
        r0 = (k % CPB) * ROWS
        pp = parts[k]
        zf = pp[:, 0:16].reshape(128, 8, 2).sum(2) + pp[:, 16:32].reshape(128, 8, 2).sum(2) / KAPPA
        zp = pp[:, 32:40]
        s = pp[:, 40:48]
        ce_rows = np.log(zf) - s / zp          # [128 part, 8 blocks]
        perm = _CACHE[f"perm{b}"]
        w = weights[b, :, 0][perm][r0 : r0 + ROWS].reshape(8, 128)  # [rb, p]
        ce[b] += (ce_rows.T * w).sum()
    f1 = np.asarray(pointfea1, np.float64)
    f2 = np.asarray(pointfea2, np.float64)
    reg = (f1[:, :, 3:] ** 2 + f2[:, :, 3:] ** 2).mean(2).mean(1)
    return ce.astype(np.float32), reg.astype(np.float32)


def kernel(points, pointfea1, pointfea2, weights):
    nc = get_nc()
    points = np.asarray(points, np.float32)
    for b in range(B):
        _CACHE[f"perm{b}"] = np.argsort(_morton(points[b]))
    in_maps = make_in_maps(points, pointfea1, pointfea2, weights)
    res = run_bass_kernel_spmd(nc, in_maps, core_ids=list(range(NCORES)))
    parts = np.stack([res.results[k]["partials"] for k in range(NCORES)])
    return combine_partials(parts, points, pointfea1, pointfea2, weights)
